# revision 4
# baseline (speedup 1.0000x reference)
"""Transformer block (LN->attn->residual->LN->MLP->residual) on 8 TRN2 cores.

Sharding (zero-collective): core c -> batch b=c//2, token-half p=c%2.
Host permutes tokens per core so "my q-half" is always positions 0:1024;
each core computes LN1 + full qkv (k,v for all 2048 tokens, q for its half),
all 16 heads of attention for its q-half, then proj/LN2/MLP token-locally.
All activations are channel-major [C, T]; v is token-major. All matmul
operands are float32r (full-rate PE, ~1e-4 rounding).

kernel(**inputs) takes the FULL unsharded inputs and returns the FULL output.
"""
import numpy as np

import concourse.bacc as bacc
import concourse.mybir as mybir
import concourse.tile as tile
from concourse import bass_utils

F32 = mybir.dt.float32
F32R = mybir.dt.float32r
AF = mybir.ActivationFunctionType
ALU = mybir.AluOpType

C = 1024
T = 2048
TH = 1024
H = 16
D = 64
HID = 4096
EPS = 1e-5
KC = 8          # C / 128
N_CORES = 8

_CACHE = {}


# --------------------------------------------------------------------------
# host-side prep
# --------------------------------------------------------------------------

def _prep_core_inputs(c, inp):
    b, p = c // 2, c % 2
    x = np.asarray(inp["x"], dtype=np.float32)[b]        # [T, C]
    if p == 1:
        x = np.concatenate([x[TH:], x[:TH]], axis=0)
    xT = np.ascontiguousarray(x.T)                       # [C, T]

    g1 = np.asarray(inp["ln1_g"], np.float32)
    b1 = np.asarray(inp["ln1_b"], np.float32)
    g2 = np.asarray(inp["ln2_g"], np.float32)
    b2 = np.asarray(inp["ln2_b"], np.float32)
    qkv_w = np.asarray(inp["qkv_w"], np.float32)
    qkv_b = np.asarray(inp["qkv_b"], np.float32)
    proj_w = np.asarray(inp["proj_w"], np.float32)
    proj_b = np.asarray(inp["proj_b"], np.float32)
    fc1_w = np.asarray(inp["fc1_w"], np.float32)
    fc1_b = np.asarray(inp["fc1_b"], np.float32)
    fc2_w = np.asarray(inp["fc2_w"], np.float32)
    fc2_b = np.asarray(inp["fc2_b"], np.float32)

    Wq = qkv_w[0:C] * g1[None, :]
    Wk = qkv_w[C:2 * C] * g1[None, :]
    Wv = qkv_w[2 * C:3 * C] * g1[None, :]
    bq = qkv_b[0:C] + qkv_w[0:C] @ b1
    bk = qkv_b[C:2 * C] + qkv_w[C:2 * C] @ b1
    bv = qkv_b[2 * C:3 * C] + qkv_w[2 * C:3 * C] @ b1
    scale = 1.0 / np.sqrt(D)
    Wq = Wq * scale
    bq = bq * scale

    w_qk = np.concatenate([Wq.T, Wk.T], axis=1)          # [1024, 2048]
    w1_qk = -w_qk.sum(axis=0, keepdims=True)
    b_qk = np.concatenate([bq, bk]).reshape(16, 128).T.copy()

    w_v = np.zeros((C, H * 65), np.float32)
    bv_row = np.zeros((1, H * 65), np.float32)
    for h in range(H):
        w_v[:, h * 65:h * 65 + 64] = Wv.T[:, h * 64:(h + 1) * 64]
        bv_row[0, h * 65:h * 65 + 64] = bv[h * 64:(h + 1) * 64]
        bv_row[0, h * 65 + 64] = 1.0
    w1_v = -w_v.sum(axis=0, keepdims=True)

    W1 = fc1_w * g2[None, :]
    b1f = fc1_b + fc1_w @ b2

    return {
        "x": xT,
        "w_qk": np.ascontiguousarray(w_qk),
        "w1_qk": np.ascontiguousarray(w1_qk),
        "b_qk": b_qk,
        "w_v": w_v,
        "w1_v": np.ascontiguousarray(w1_v),
        "bv_row": bv_row,
        "w_p": np.ascontiguousarray(proj_w.T),
        "b_p": proj_b.reshape(8, 128).T.copy(),
        "w_1": np.ascontiguousarray(W1.T),
        "w1_1": np.ascontiguousarray(-W1.T.sum(axis=0, keepdims=True)),
        "b_1": b1f.reshape(32, 128).T.copy(),
        "w_2": np.ascontiguousarray(fc2_w.T),
        "b_2": fc2_b.reshape(8, 128).T.copy(),
        "onesC": np.full((128, 128), 1.0 / C, np.float32),
        "ident": np.eye(128, dtype=np.float32),
        "ones1": np.ones((1, T), np.float32),
    }


# --------------------------------------------------------------------------
# device program
# --------------------------------------------------------------------------

def _mm(nc, out, lhsT, rhs, start, stop, maxn=512):
    """matmul split into <=512-wide moving chunks."""
    n = rhs.shape[-1]
    ofs = 0
    while ofs < n:
        w = min(maxn, n - ofs)
        nc.tensor.matmul(out[:, ofs:ofs + w], lhsT, rhs[:, ofs:ofs + w],
                         start=start, stop=stop)
        ofs += w


def build_nc():
    nc = bacc.Bacc("TRN2", target_bir_lowering=False, debug=False,
                   num_devices=N_CORES)

    def din(name, shape, dt=F32R):
        return nc.dram_tensor(name, shape, dt, kind="ExternalInput").ap()

    X = din("x", [C, T])
    WQK = din("w_qk", [C, 2 * C])
    W1QK = din("w1_qk", [1, 2 * C])
    BQK = din("b_qk", [128, 16], F32)
    WV = din("w_v", [C, H * 65])
    W1V = din("w1_v", [1, H * 65])
    BVR = din("bv_row", [1, H * 65])
    WP = din("w_p", [C, C])
    BP = din("b_p", [128, 8], F32)
    W1 = din("w_1", [C, HID])
    W11 = din("w1_1", [1, HID])
    B1 = din("b_1", [128, 32], F32)
    W2 = din("w_2", [HID, C])
    B2 = din("b_2", [128, 8], F32)
    ONESC = din("onesC", [128, 128])
    IDENT = din("ident", [128, 128])
    ONES1 = din("ones1", [1, T])
    OUT = nc.dram_tensor("out", [C, TH], F32, kind="ExternalOutput").ap()

    with tile.TileContext(nc) as tc:
        with tc.tile_pool(name="consts", bufs=1) as cp, \
             tc.tile_pool(name="dram", bufs=1, space="DRAM") as dram:
            onesC = cp.tile([128, 128], F32R, name="onesC_sb")
            ident = cp.tile([128, 128], F32R, name="ident_sb")
            ones1 = cp.tile([1, T], F32R, name="ones1_sb")
            b_qk = cp.tile([128, 16], F32, name="b_qk_sb")
            b_p = cp.tile([128, 8], F32, name="b_p_sb")
            b_1 = cp.tile([128, 32], F32, name="b_1_sb")
            b_2 = cp.tile([128, 8], F32, name="b_2_sb")
            w1_qk = cp.tile([1, 2 * C], F32R, name="w1_qk_sb")
            w1_v = cp.tile([1, H * 65], F32R, name="w1_v_sb")
            bv_row = cp.tile([1, H * 65], F32R, name="bv_row_sb")
            eps_t = cp.tile([128, 1], F32, name="eps_sb")
            nc.gpsimd.memset(eps_t, EPS)
            for t, s in [(onesC, ONESC), (ident, IDENT), (ones1, ONES1),
                         (b_qk, BQK), (b_p, BP), (b_1, B1), (b_2, B2),
                         (w1_qk, W1QK), (w1_v, W1V), (bv_row, BVR)]:
                nc.sync.dma_start(t, s)

            q_d = dram.tile([C, TH], F32R, name="q_d")
            k_d = dram.tile([C, T], F32R, name="k_d")
            v_d = dram.tile([T, H * 65], F32R, name="v_d")
            yn_d = dram.tile([C, TH], F32R, name="yn_d")

            # ============================ PHASE A ============================
            with tc.tile_pool(name="pA", bufs=1) as pA:
                # --- stats
                with tc.tile_pool(name="psS", bufs=1, space="PSUM") as psS:
                    ps_mean = psS.tile([128, T], F32, name="ps_mean")
                    ps_m2 = psS.tile([128, T], F32, name="ps_m2")
                    for kc in range(KC):
                        xc = pA.tile([128, T], F32R, tag="xc", bufs=2,
                                     name=f"xcA{kc}")
                        nc.sync.dma_start(xc, X[kc * 128:(kc + 1) * 128, :])
                        sq = pA.tile([128, T], F32R, tag="sq", bufs=2,
                                     name=f"sqA{kc}")
                        nc.gpsimd.tensor_tensor(sq, xc, xc, ALU.mult)
                        _mm(nc, ps_mean, onesC, xc, kc == 0, kc == KC - 1)
                        _mm(nc, ps_m2, onesC, sq, kc == 0, kc == KC - 1)
                    t1 = pA.tile([128, T], F32, tag="sq", bufs=2, name="t1")
                    nc.scalar.activation(t1, ps_mean, AF.Square, bias=0.0,
                                         scale=1.0)
                    var = pA.tile([128, T], F32, tag="sq", bufs=2, name="var")
                    nc.vector.tensor_tensor(var, ps_m2, t1, ALU.subtract)
                    lnv = pA.tile([128, T], F32, tag="sq", bufs=2, name="lnv")
                    nc.scalar.activation(lnv, var, AF.Ln, bias=eps_t, scale=1.0)
                    rs_b = pA.tile([128, T], F32, tag="rs", name="rs_b")
                    nc.scalar.activation(rs_b, lnv, AF.Exp, bias=0.0, scale=-0.5)
                    murs = pA.tile([1, T], F32R, tag="murs", name="murs")
                    nc.vector.tensor_tensor(murs, ps_mean[0:1, :], rs_b[0:1, :],
                                            ALU.mult)

                # --- xs = x * rs
                xs = []
                for kc in range(KC):
                    xc2 = pA.tile([128, T], F32R, tag="xc", bufs=2,
                                  name=f"xcB{kc}")
                    nc.sync.dma_start(xc2, X[kc * 128:(kc + 1) * 128, :])
                    xst = pA.tile([128, T], F32R, tag=f"xs{kc}", name=f"xs{kc}")
                    nc.vector.tensor_tensor(xst, xc2, rs_b, ALU.mult)
                    xs.append(xst)

                # --- q (my half)
                wq = []
                for kc in range(KC):
                    w = pA.tile([128, H * 65], F32R, tag="w", bufs=8,
                                name=f"wq{kc}")
                    nc.sync.dma_start(w[:, 0:C], WQK[kc * 128:(kc + 1) * 128, 0:C])
                    wq.append(w)
                with tc.tile_pool(name="psQ", bufs=1, space="PSUM") as psQ:
                    for mt in range(KC):
                        pq = psQ.tile([128, TH], F32, tag="pq", bufs=2,
                                      name=f"pq{mt}")
                        for kc in range(KC):
                            _mm(nc, pq, wq[kc][:, mt * 128:(mt + 1) * 128],
                                xs[kc][:, 0:TH], kc == 0, False)
                        _mm(nc, pq, w1_qk[0:1, mt * 128:(mt + 1) * 128],
                            murs[0:1, 0:TH], False, True)
                        qdr = pA.tile([128, TH], F32R, tag="xc", bufs=2,
                                      name=f"qdr{mt}")
                        nc.scalar.activation(qdr, pq, AF.Identity,
                                             bias=b_qk[:, mt:mt + 1], scale=1.0)
                        nc.sync.dma_start(q_d[mt * 128:(mt + 1) * 128, :], qdr)

                # --- k (all tokens)
                wk = []
                for kc in range(KC):
                    w = pA.tile([128, H * 65], F32R, tag="w", bufs=8,
                                name=f"wk{kc}")
                    nc.sync.dma_start(w[:, 0:C],
                                      WQK[kc * 128:(kc + 1) * 128, C:2 * C])
                    wk.append(w)
                with tc.tile_pool(name="psK", bufs=1, space="PSUM") as psK:
                    for mt in range(KC):
                        pk = psK.tile([128, T], F32, tag="pk", bufs=2,
                                      name=f"pk{mt}")
                        for kc in range(KC):
                            _mm(nc, pk, wk[kc][:, mt * 128:(mt + 1) * 128],
                                xs[kc], kc == 0, False)
                        _mm(nc, pk, w1_qk[0:1, C + mt * 128:C + (mt + 1) * 128],
                            murs, False, True)
                        kdr = pA.tile([128, T], F32R, tag="xc", bufs=2,
                                      name=f"kdr{mt}")
                        nc.scalar.activation(kdr, pk, AF.Identity,
                                             bias=b_qk[:, 8 + mt:9 + mt],
                                             scale=1.0)
                        nc.sync.dma_start(k_d[mt * 128:(mt + 1) * 128, :], kdr)

                # --- v (token-major, aug)
                wv = []
                for kc in range(KC):
                    w = pA.tile([128, H * 65], F32R, tag="w", bufs=8,
                                name=f"wv{kc}")
                    nc.sync.dma_start(w, WV[kc * 128:(kc + 1) * 128, :])
                    wv.append(w)
                with tc.tile_pool(name="psV", bufs=1, space="PSUM") as psV:
                    for tt in range(T // 128):
                        pv = psV.tile([128, H * 65], F32, tag="pv", bufs=2,
                                      name=f"pv{tt}")
                        sl = slice(tt * 128, (tt + 1) * 128)
                        for kc in range(KC):
                            _mm(nc, pv, xs[kc][:, sl], wv[kc], kc == 0, False)
                        _mm(nc, pv, murs[0:1, sl], w1_v, False, False)
                        _mm(nc, pv, ones1[0:1, sl], bv_row, False, True)
                        vdr = pA.tile([128, H * 65], F32R, tag="xc", bufs=2,
                                      name=f"vdr{tt}")
                        nc.scalar.copy(vdr, pv)
                        nc.sync.dma_start(v_d[sl, :], vdr)

            # ============================ PHASE B ============================
            with tc.tile_pool(name="pB", bufs=1) as pB:
                den16 = pB.tile([16, TH], F32, tag="den", name="den16")
                yraws = []
                with tc.tile_pool(name="psB", bufs=1, space="PSUM") as psB:
                    for g in range(8):
                        qp = pB.tile([128, TH], F32R, tag="qp", bufs=2,
                                     name=f"qp{g}")
                        nc.sync.dma_start(qp, q_d[g * 128:(g + 1) * 128, :])
                        kp = pB.tile([128, T], F32R, tag="kp", bufs=2,
                                     name=f"kp{g}")
                        nc.sync.dma_start(kp, k_d[g * 128:(g + 1) * 128, :])
                        vp = pB.tile([128, 16 * 130], F32R, tag="vp", bufs=2,
                                     name=f"vp{g}")
                        nc.sync.dma_start(
                            vp,
                            v_d[:, g * 130:(g + 1) * 130].rearrange(
                                "(tt p) c -> p tt c", p=128))
                        py1 = psB.tile([65, TH], F32, tag="y1", name=f"py1_{g}")
                        py2 = psB.tile([65, TH], F32, tag="y2", name=f"py2_{g}")
                        for kc in range(16):
                            ksl = slice(kc * 128, (kc + 1) * 128)
                            pscore = psB.tile([128, T], F32, tag="score",
                                              name=f"sc{g}_{kc}")
                            _mm(nc, pscore[:, 0:TH], kp[0:64, ksl], qp[0:64, :],
                                True, True)
                            _mm(nc, pscore[:, TH:T], kp[64:128, ksl],
                                qp[64:128, :], True, True)
                            e = pB.tile([128, T], F32R, tag="e", bufs=3,
                                        name=f"e{g}_{kc}")
                            nc.scalar.activation(e, pscore, AF.Exp, bias=0.0,
                                                 scale=1.0)
                            _mm(nc, py1, vp[:, kc * 130:kc * 130 + 65],
                                e[:, 0:TH], kc == 0, kc == 15)
                            _mm(nc, py2, vp[:, kc * 130 + 65:(kc + 1) * 130],
                                e[:, TH:T], kc == 0, kc == 15)
                        yr1 = pB.tile([65, TH], F32, tag="yraw", bufs=16,
                                      name=f"yr{2 * g}")
                        nc.vector.tensor_copy(yr1, py1)
                        yr2 = pB.tile([65, TH], F32, tag="yraw", bufs=16,
                                      name=f"yr{2 * g + 1}")
                        nc.vector.tensor_copy(yr2, py2)
                        nc.sync.dma_start(den16[2 * g:2 * g + 1, :],
                                          yr1[64:65, :])
                        nc.sync.dma_start(den16[2 * g + 1:2 * g + 2, :],
                                          yr2[64:65, :])
                        yraws += [yr1, yr2]

                # normalize: recip of denominators, broadcast, multiply
                resh = pB.tile([128, 128], F32, tag="resh", name="resh")
                nc.sync.dma_start(resh, den16.rearrange("a (p f) -> a p f", p=8))
                rec = pB.tile([128, 128], F32, tag="rec", name="rec")
                nc.vector.reciprocal(rec, resh)
                with tc.tile_pool(name="psN", bufs=1, space="PSUM") as psN:
                    for h in range(H):
                        rh = pB.tile([1, TH], F32R, tag="rech", bufs=2,
                                     name=f"rech{h}")
                        nc.sync.dma_start(rh, rec[h * 8:(h + 1) * 8, :]
                                          .bitcast(F32R))
                        pb = psN.tile([64, TH], F32, tag="bc", bufs=2,
                                      name=f"pb{h}")
                        _mm(nc, pb, ones1[0:1, 0:64], rh,
                            True, True)
                        ynh = pB.tile([64, TH], F32R, tag="ynh", bufs=2,
                                      name=f"ynh{h}")
                        nc.vector.tensor_tensor(ynh, yraws[h][0:64, :], pb,
                                                ALU.mult)
                        nc.sync.dma_start(yn_d[h * 64:(h + 1) * 64, :], ynh)

            # ============================ PHASE C ============================
            with tc.tile_pool(name="pC", bufs=1) as pC:
                x2 = []
                # --- proj + residual
                with tc.tile_pool(name="pP", bufs=1) as pP, \
                     tc.tile_pool(name="psP", bufs=1, space="PSUM") as psP:
                    wp = []
                    ync = []
                    for kc in range(KC):
                        w = pP.tile([128, C], F32R, tag="wp", bufs=8,
                                    name=f"wp{kc}")
                        nc.sync.dma_start(w, WP[kc * 128:(kc + 1) * 128, :])
                        wp.append(w)
                        y = pP.tile([128, TH], F32R, tag="ync", bufs=8,
                                    name=f"ync{kc}")
                        nc.sync.dma_start(y, yn_d[kc * 128:(kc + 1) * 128, :])
                        ync.append(y)
                    for mt in range(KC):
                        pp = psP.tile([128, TH], F32, tag="pp", bufs=2,
                                      name=f"pp{mt}")
                        for kc in range(KC):
                            _mm(nc, pp, wp[kc][:, mt * 128:(mt + 1) * 128],
                                ync[kc], kc == 0, False)
                        xh = pP.tile([128, TH], F32R, tag="xh", bufs=2,
                                     name=f"xh{mt}")
                        nc.sync.dma_start(xh, X[mt * 128:(mt + 1) * 128, 0:TH])
                        _mm(nc, pp, ident, xh, False, True)
                        x2t = pC.tile([128, TH], F32R, tag=f"x2_{mt}",
                                      name=f"x2_{mt}")
                        nc.scalar.activation(x2t, pp, AF.Identity,
                                             bias=b_p[:, mt:mt + 1], scale=1.0)
                        x2.append(x2t)

                # --- LN2
                xs2 = []
                with tc.tile_pool(name="pL", bufs=1) as pL, \
                     tc.tile_pool(name="psL", bufs=1, space="PSUM") as psL:
                    ps_mean2 = psL.tile([128, TH], F32, name="ps_mean2")
                    ps_m22 = psL.tile([128, TH], F32, name="ps_m22")
                    for kc in range(KC):
                        sq2 = pL.tile([128, TH], F32R, tag="sq2", bufs=2,
                                      name=f"sq2_{kc}")
                        nc.gpsimd.tensor_tensor(sq2, x2[kc], x2[kc], ALU.mult)
                        _mm(nc, ps_mean2, onesC, x2[kc], kc == 0, kc == KC - 1)
                        _mm(nc, ps_m22, onesC, sq2, kc == 0, kc == KC - 1)
                    t12 = pL.tile([128, TH], F32, tag="stat2", bufs=2,
                                  name="t12")
                    nc.scalar.activation(t12, ps_mean2, AF.Square, bias=0.0,
                                         scale=1.0)
                    var2 = pL.tile([128, TH], F32, tag="stat2", bufs=2,
                                   name="var2")
                    nc.vector.tensor_tensor(var2, ps_m22, t12, ALU.subtract)
                    lnv2 = pL.tile([128, TH], F32, tag="stat2", bufs=2,
                                   name="lnv2")
                    nc.scalar.activation(lnv2, var2, AF.Ln, bias=eps_t,
                                         scale=1.0)
                    rs2 = pL.tile([128, TH], F32, tag="rs2", name="rs2")
                    nc.scalar.activation(rs2, lnv2, AF.Exp, bias=0.0,
                                         scale=-0.5)
                    murs2 = pC.tile([1, TH], F32R, tag="murs2", name="murs2")
                    nc.vector.tensor_tensor(murs2, ps_mean2[0:1, :],
                                            rs2[0:1, :], ALU.mult)
                    for kc in range(KC):
                        xst2 = pC.tile([128, TH], F32R, tag=f"xs2_{kc}",
                                       name=f"xs2_{kc}")
                        nc.vector.tensor_tensor(xst2, x2[kc], rs2, ALU.mult)
                        xs2.append(xst2)

                # --- MLP (2 token sub-blocks of 512)
                with tc.tile_pool(name="pM", bufs=1) as pM, \
                     tc.tile_pool(name="ps1", bufs=1, space="PSUM") as ps1, \
                     tc.tile_pool(name="ps2", bufs=1, space="PSUM") as ps2:
                    w1_1 = pM.tile([1, HID], F32R, tag="w1_1", name="w1_1_sb")
                    nc.sync.dma_start(w1_1, W11)
                    for sbk in range(2):
                        tok = slice(sbk * 512, (sbk + 1) * 512)
                        h1 = []
                        for mt in range(32):
                            w1b = pM.tile([128, C], F32R, tag="w1b", bufs=3,
                                          name=f"w1b_{sbk}_{mt}")
                            nc.sync.dma_start(
                                w1b,
                                W1[:, mt * 128:(mt + 1) * 128].rearrange(
                                    "(kc p) m -> p kc m", p=128))
                            p1 = ps1.tile([128, 512], F32, tag="p1", bufs=3,
                                          name=f"p1_{sbk}_{mt}")
                            for kc in range(KC):
                                _mm(nc, p1, w1b[:, kc * 128:(kc + 1) * 128],
                                    xs2[kc][:, tok], kc == 0, False)
                            _mm(nc, p1, w1_1[0:1, mt * 128:(mt + 1) * 128],
                                murs2[0:1, tok], False, True)
                            h1t = pM.tile([128, 512], F32R, tag=f"h1_{mt}",
                                          name=f"h1_{sbk}_{mt}")
                            nc.scalar.activation(h1t, p1, AF.Gelu,
                                                 bias=b_1[:, mt:mt + 1],
                                                 scale=1.0)
                            h1.append(h1t)
                        for mt2 in range(KC):
                            p2 = ps2.tile([128, 512], F32, tag="p2", bufs=3,
                                          name=f"p2_{sbk}_{mt2}")
                            for half in range(2):
                                w2b = pM.tile([128, HID // 2], F32R, tag="w2b",
                                              bufs=2, name=f"w2b_{sbk}_{mt2}_{half}")
                                nc.sync.dma_start(
                                    w2b,
                                    W2[half * 2048:(half + 1) * 2048,
                                       mt2 * 128:(mt2 + 1) * 128].rearrange(
                                        "(kc p) m -> p kc m", p=128))
                                for k2 in range(16):
                                    kc2 = half * 16 + k2
                                    _mm(nc, p2, w2b[:, k2 * 128:(k2 + 1) * 128],
                                        h1[kc2], kc2 == 0, False)
                            _mm(nc, p2, ident, x2[mt2][:, tok], False, True)
                            odr = pM.tile([128, 512], F32, tag="odr", bufs=2,
                                          name=f"odr_{sbk}_{mt2}")
                            nc.scalar.activation(odr, p2, AF.Identity,
                                                 bias=b_2[:, mt2:mt2 + 1],
                                                 scale=1.0)
                            nc.sync.dma_start(
                                OUT[mt2 * 128:(mt2 + 1) * 128, tok], odr)

    nc.compile()
    return nc


# --------------------------------------------------------------------------
# entry point
# --------------------------------------------------------------------------

def kernel(**inputs):
    nc = _CACHE.get("nc")
    if nc is None:
        nc = build_nc()
        _CACHE["nc"] = nc
    in_maps = [_prep_core_inputs(c, inputs) for c in range(N_CORES)]
    res = bass_utils.run_bass_kernel_spmd(nc, in_maps,
                                          core_ids=list(range(N_CORES)))
    full = np.zeros((4, T, C), np.float32)
    for c in range(N_CORES):
        b, p = c // 2, c % 2
        full[b, p * TH:(p + 1) * TH, :] = res.results[c]["out"].T
    return full
